# revision 6
# baseline (speedup 1.0000x reference)
"""AdaptiveTokenSampling on 8 TRN2 NeuronCores (Bass/Tile, batch-parallel).

Per-core (one batch element):
  1. score pipeline: value norms + cls attention -> pseudo-logits (token-partition layout)
  2. gumbel argmax sampling (vector.max/max_index) -> 256 sampled token ids
  3. sort-based unique via comparison-matrix rank/scatter (PE matmuls on 0/1 matrices)
  4. indirect-DMA gather of attn rows -> new_attn
"""
import numpy as np

import concourse.bacc as bacc
import concourse.bass as bass
import concourse.mybir as mybir
import concourse.tile as tile
from concourse.bass import IndirectOffsetOnAxis
from concourse.bass_utils import run_bass_kernel_spmd

F32 = mybir.dt.float32
I32 = mybir.dt.int32
U32 = mybir.dt.uint32
U8 = mybir.dt.uint8

B, H, N, D, K = 8, 12, 1025, 64, 256
NM1 = N - 1          # 1024
NH = N * H           # 12300 rows in flattened attn
KP1 = K + 1          # 257
EPS = 1e-6
MASKVAL = float(-np.finfo(np.float32).max / 2)
DUMP = 300.0         # parking position for non-survivor elements (>256)


def _build():
    nc = bacc.Bacc(None, target_bir_lowering=False, debug=False, num_devices=8)

    attn_d = nc.declare_dram_parameter("attn", [NH, N], F32, isOutput=False)
    val_d = nc.declare_dram_parameter("value", [H, N * D], F32, isOutput=False)
    msk_d = nc.declare_dram_parameter("maskp", [N], U8, isOutput=False)
    gum_d = nc.declare_dram_parameter("gumbel", [K, NM1], F32, isOutput=False)

    oattn_d = nc.declare_dram_parameter("out_attn", [H, KP1, N], F32, isOutput=True)
    oids_d = nc.declare_dram_parameter("out_ids", [KP1], I32, isOutput=True)
    omask_d = nc.declare_dram_parameter("out_mask", [KP1], U8, isOutput=True)

    pl_dram = nc.dram_tensor("pl_dram", [NM1], F32)
    sc_dram = nc.dram_tensor("sc_dram", [1], F32)

    ident_c = nc.inline_tensor(np.eye(128, dtype=np.float32), name="ident_c")
    iota128_c = nc.inline_tensor(
        np.broadcast_to(np.arange(128, dtype=np.float32), (128, 128)).copy(), name="iota128_c")
    # ILT_a[p, i] = 1.0 if (global source index p+128a) < i else 0.0
    gi = np.arange(K, dtype=np.float32)
    ilt0_c = nc.inline_tensor(
        (np.arange(128, dtype=np.float32)[:, None] < gi[None, :]).astype(np.float32), name="ilt0_c")
    ilt1_c = nc.inline_tensor(
        ((np.arange(128, dtype=np.float32) + 128.0)[:, None] < gi[None, :]).astype(np.float32),
        name="ilt1_c")
    hoff_c = nc.inline_tensor(
        (np.arange(H, dtype=np.float32) * float(N))[:, None].copy(), name="hoff_c")

    with tile.TileContext(nc) as tc:
        with (
            tc.tile_pool(name="const", bufs=1) as cp,
            tc.tile_pool(name="big", bufs=1) as bigp,
            tc.tile_pool(name="work", bufs=2) as wp,
            tc.tile_pool(name="keep", bufs=1) as kp,
            tc.tile_pool(name="ps", bufs=2, space="PSUM") as ps,
            tc.tile_pool(name="psacc", bufs=2, space="PSUM") as psa,
            tc.tile_pool(name="gath", bufs=6) as gp,
        ):
            # ---- constants ----
            ident = cp.tile([128, 128], F32)
            nc.sync.dma_start(out=ident[:], in_=ident_c[:])
            iota128 = cp.tile([128, 128], F32)
            nc.sync.dma_start(out=iota128[:], in_=iota128_c[:])
            ilt = []
            for a, src in enumerate((ilt0_c, ilt1_c)):
                t = cp.tile([128, K], F32, tag=f"ilt{a}")
                nc.sync.dma_start(out=t[:], in_=src[:])
                ilt.append(t)
            hoff = cp.tile([H, 1], F32)
            nc.sync.dma_start(out=hoff[:], in_=hoff_c[:])
            ones_col = cp.tile([128, 1], F32)
            nc.vector.memset(ones_col[:], 1.0)
            one_cell = cp.tile([1, 1], F32)
            nc.vector.memset(one_cell[:], 1.0)
            eps_col = cp.tile([128, 1], F32)
            nc.vector.memset(eps_col[:], EPS)

            # PE warmup: observe the const DMAs once so PE-transposes need one new wait
            warm = ps.tile([1, 1], F32, tag="small")
            nc.tensor.matmul(warm[:], lhsT=ident[:, 0:1], rhs=iota128[:, 0:1],
                             start=True, stop=True)

            # ---- stage 1: scores (token-partition layout: token j-1 = 8p + c) ----
            # value[h, 1:, :] for all h as [128, (h, 8tok, 64)]
            vt = bigp.tile([128, H * 512], F32)
            nc.sync.dma_start(
                out=vt[:].rearrange("p (h f) -> p h f", f=512),
                in_=val_d[:][:, D:].rearrange("h (p f) -> p h f", f=512),
            )
            sq = bigp.tile([128, H * 512], F32)
            nc.scalar.square(sq[:], vt[:])
            norms2 = kp.tile([128, H * 8], F32)
            nc.vector.tensor_reduce(
                out=norms2[:], in_=sq[:].rearrange("p (g d) -> p g d", d=D),
                axis=mybir.AxisListType.X, op=mybir.AluOpType.add)
            norms = kp.tile([128, H * 8], F32)
            nc.scalar.sqrt(norms[:], norms2[:])

            cls = kp.tile([128, H * 8], F32)
            nc.sync.dma_start(
                out=cls[:].rearrange("p (h c) -> p h c", c=8),
                in_=bass.AP(attn_d, 1, [[8, 128], [N * N, H], [1, 8]]),
            )
            prod = kp.tile([128, H * 8], F32)
            nc.vector.tensor_mul(prod[:], cls[:], norms[:])
            score = kp.tile([128, 8], F32)
            nc.vector.tensor_reduce(
                out=score[:], in_=prod[:].rearrange("p (h c) -> p c h", c=8),
                axis=mybir.AxisListType.X, op=mybir.AluOpType.add)

            sumrow = wp.tile([128, 1], F32)
            nc.vector.tensor_reduce(out=sumrow[:], in_=score[:],
                                    axis=mybir.AxisListType.X, op=mybir.AluOpType.add)
            total_ps = ps.tile([1, 1], F32, tag="small")
            nc.tensor.matmul(total_ps[:], lhsT=sumrow[:], rhs=ones_col[:],
                             start=True, stop=True)
            total = wp.tile([1, 1], F32)
            nc.vector.tensor_scalar(total[:], total_ps[:], EPS, None, op0=mybir.AluOpType.add)
            recip = wp.tile([1, 1], F32)
            nc.vector.reciprocal(recip[:], total[:])
            # broadcast recip to 128 partitions via DRAM replicate
            nc.sync.dma_start(out=sc_dram[:][None, :], in_=recip[:])
            recipB = wp.tile([128, 1], F32)
            nc.sync.dma_start(out=recipB[:], in_=bass.AP(sc_dram, 0, [[0, 128], [1, 1]]))

            pl = kp.tile([128, 8], F32)
            nc.scalar.activation(pl[:], score[:], mybir.ActivationFunctionType.Ln,
                                 bias=eps_col[:, 0:1], scale=recipB[:, 0:1])
            # mask (ones in practice, but implement reference semantics exactly)
            mku = wp.tile([128, 8], U8)
            nc.sync.dma_start(out=mku[:], in_=msk_d[:][None, 1:].rearrange("o (p c) -> (o p) c", c=8))
            mkf = wp.tile([128, 8], F32)
            nc.vector.tensor_copy(mkf[:], mku[:])
            plm = kp.tile([128, 8], F32)
            nc.vector.tensor_mul(plm[:], pl[:], mkf[:])
            inv = wp.tile([128, 8], F32)
            nc.vector.tensor_scalar(inv[:], mkf[:], 0.5, None, op0=mybir.AluOpType.is_lt)
            nc.vector.tensor_scalar(inv[:], inv[:], MASKVAL, None, op0=mybir.AluOpType.mult)
            nc.vector.tensor_add(plm[:], plm[:], inv[:])
            # flatten to DRAM in token order (partition-major), then used replicated
            nc.sync.dma_start(out=pl_dram[:].rearrange("(p c) -> p c", c=8), in_=plm[:])

            # ---- stage 2: gumbel argmax sampling ----
            ids_col = []
            for a in range(2):
                gt = wp.tile([128, NM1], F32, tag="gt")
                nc.sync.dma_start(out=gt[:], in_=gum_d[:][a * 128:(a + 1) * 128, :])
                nc.gpsimd.dma_start(out=gt[:], in_=bass.AP(pl_dram, 0, [[0, 128], [1, NM1]]),
                                    accum_op=mybir.AluOpType.add)
                mx8 = wp.tile([128, 8], F32, tag="mx8")
                nc.vector.max(out=mx8[:], in_=gt[:])
                ix8 = wp.tile([128, 8], U32, tag="ix8")
                nc.vector.max_index(out=ix8[:], in_max=mx8[:], in_values=gt[:])
                ixf = wp.tile([128, 1], F32, tag="ixf")
                nc.vector.tensor_copy(ixf[:], ix8[:, 0:1])
                idc = kp.tile([128, 1], F32, tag=f"idc{a}")
                nc.vector.tensor_scalar(idc[:], ixf[:], 1.0, None, op0=mybir.AluOpType.add)
                ids_col.append(idc)

            # ---- stage 3: unique + sorted positions (all values are exact small ints in f32) ----
            idsT = kp.tile([128, K], F32)
            for a in range(2):
                tp = ps.tile([128, 128], F32, tag="tp")
                nc.tensor.transpose(tp[:], ids_col[a][:].to_broadcast([128, 128]), ident[:])
                nc.vector.tensor_copy(idsT[:, a * 128:(a + 1) * 128], tp[:])

            LT, EE = [], []
            for a in range(2):
                lt = kp.tile([128, K], F32, tag=f"lt{a}")
                nc.vector.tensor_tensor(lt[:], ids_col[a][:].to_broadcast([128, K]), idsT[:],
                                        op=mybir.AluOpType.is_lt)
                eq = wp.tile([128, K], F32, tag="eq")
                nc.vector.tensor_tensor(eq[:], ids_col[a][:].to_broadcast([128, K]), idsT[:],
                                        op=mybir.AluOpType.is_equal)
                ee = wp.tile([128, K], F32, tag=f"ee{a}")
                nc.vector.tensor_mul(ee[:], eq[:], ilt[a][:])
                LT.append(lt)
                EE.append(ee)

            eqc_ps = psa.tile([1, K], F32, tag="acc")
            for a in range(2):
                nc.tensor.matmul(eqc_ps[:], lhsT=ones_col[:], rhs=EE[a][:],
                                 start=(a == 0), stop=(a == 1))
            eqc = kp.tile([1, K], F32)
            nc.vector.tensor_copy(eqc[:], eqc_ps[:])

            F_col = []
            for a in range(2):
                cps = ps.tile([128, 1], F32, tag="small")
                nc.tensor.matmul(cps[:], lhsT=eqc[0:1, a * 128:(a + 1) * 128], rhs=one_cell[:],
                                 start=True, stop=True)
                fc = kp.tile([128, 1], F32, tag=f"fc{a}")
                nc.vector.tensor_scalar(fc[:], cps[:], 0.0, None, op0=mybir.AluOpType.is_equal)
                F_col.append(fc)

            pos_ps = psa.tile([1, K], F32, tag="acc")
            for a in range(2):
                nc.tensor.matmul(pos_ps[:], lhsT=F_col[a][:], rhs=LT[a][:],
                                 start=(a == 0), stop=(a == 1))
            pos = kp.tile([1, K], F32)
            nc.vector.tensor_copy(pos[:], pos_ps[:])

            OH, RHS = [], []
            for a in range(2):
                pcp = ps.tile([128, 1], F32, tag="small")
                nc.tensor.matmul(pcp[:], lhsT=pos[0:1, a * 128:(a + 1) * 128], rhs=one_cell[:],
                                 start=True, stop=True)
                # pos1 = (pos+1) if survivor else DUMP
                p1 = wp.tile([128, 1], F32, tag="p1")
                nc.vector.tensor_scalar(p1[:], pcp[:], 1.0, None, op0=mybir.AluOpType.add)
                nc.vector.tensor_mul(p1[:], p1[:], F_col[a][:])
                nsv = wp.tile([128, 1], F32, tag="nsv")
                nc.vector.tensor_scalar(nsv[:], F_col[a][:], 0.5, None, op0=mybir.AluOpType.is_lt)
                nc.vector.tensor_scalar(nsv[:], nsv[:], DUMP, None, op0=mybir.AluOpType.mult)
                nc.vector.tensor_add(p1[:], p1[:], nsv[:])
                # div = (p1>=128)+(p1>=256) in {0,1,2}; mod = p1 - 128*div
                d1 = wp.tile([128, 1], F32, tag="d1")
                nc.vector.tensor_scalar(d1[:], p1[:], 128.0, None, op0=mybir.AluOpType.is_ge)
                d2 = wp.tile([128, 1], F32, tag="d2")
                nc.vector.tensor_scalar(d2[:], p1[:], 256.0, None, op0=mybir.AluOpType.is_ge)
                dv = wp.tile([128, 1], F32, tag="dv")
                nc.vector.tensor_add(dv[:], d1[:], d2[:])
                md = wp.tile([128, 1], F32, tag="md")
                nc.vector.tensor_scalar(md[:], dv[:], 128.0, None, op0=mybir.AluOpType.mult)
                nc.vector.tensor_sub(md[:], p1[:], md[:])
                oh = kp.tile([128, 128], F32, tag=f"oh{a}")
                nc.vector.tensor_tensor(oh[:], md[:].to_broadcast([128, 128]), iota128[:],
                                        op=mybir.AluOpType.is_equal)
                # rhs cols: value * (div==c) for c in 0,1,2
                dv0 = wp.tile([128, 1], F32, tag="dv0")
                nc.vector.tensor_scalar(dv0[:], p1[:], 128.0, None, op0=mybir.AluOpType.is_lt)
                dv1 = wp.tile([128, 1], F32, tag="dv1")
                nc.vector.tensor_sub(dv1[:], d1[:], d2[:])
                rhs = kp.tile([128, 3], F32, tag=f"rhs{a}")
                nc.vector.tensor_mul(rhs[:, 0:1], ids_col[a][:], dv0[:])
                nc.vector.tensor_mul(rhs[:, 1:2], ids_col[a][:], dv1[:])
                nc.vector.tensor_mul(rhs[:, 2:3], ids_col[a][:], d2[:])
                OH.append(oh)
                RHS.append(rhs)

            vals_ps = psa.tile([128, 3], F32, tag="acc")
            for a in range(2):
                nc.tensor.matmul(vals_ps[:], lhsT=OH[a][:], rhs=RHS[a][:],
                                 start=(a == 0), stop=(a == 1))
            vals = kp.tile([128, 3], F32)  # (x, c) -> unique_ids[128c + x]; col2 only x=0 valid
            nc.vector.tensor_copy(vals[:], vals_ps[:])

            # ---- stage 4: ids / mask outputs ----
            idsi = kp.tile([128, 3], I32)
            nc.vector.tensor_copy(idsi[:], vals[:])
            nc.sync.dma_start(out=oids_d[:][0:128, None], in_=idsi[:, 0:1])
            nc.sync.dma_start(out=oids_d[:][128:256, None], in_=idsi[:, 1:2])
            nc.sync.dma_start(out=oids_d[:][256:257, None], in_=idsi[0:1, 2:3])

            mcol = kp.tile([128, 3], U8)
            nc.vector.tensor_scalar(mcol[:], vals[:], 0.0, None, op0=mybir.AluOpType.not_equal)
            nc.vector.memset(mcol[0:1, 0:1], 1)
            nc.sync.dma_start(out=omask_d[:][0:128, None], in_=mcol[:, 0:1])
            nc.sync.dma_start(out=omask_d[:][128:256, None], in_=mcol[:, 1:2])
            nc.sync.dma_start(out=omask_d[:][256:257, None], in_=mcol[0:1, 2:3])

            # ---- stage 5: gather attn rows ----
            idxf = kp.tile([128, 2 * H], F32)
            for h in range(H):
                nc.vector.tensor_scalar(idxf[:, 2 * h:2 * h + 2], vals[:, 0:2], float(N * h),
                                        None, op0=mybir.AluOpType.add)
            idxi = kp.tile([128, 2 * H], I32)
            nc.vector.tensor_copy(idxi[:], idxf[:])

            # last row (q=256): one 12-index gather; ids[256] broadcast via DRAM replicate
            nc.sync.dma_start(out=sc_dram[:][None, :], in_=vals[0:1, 2:3])
            lastb = wp.tile([H, 1], F32, tag="lastb")
            nc.sync.dma_start(out=lastb[:], in_=bass.AP(sc_dram, 0, [[0, H], [1, 1]]))
            lastf = wp.tile([H, 1], F32, tag="lastf")
            nc.vector.tensor_add(lastf[:], lastb[:], hoff[:])
            lasti = kp.tile([H, 1], I32)
            nc.vector.tensor_copy(lasti[:], lastf[:])

            for h in range(H):
                for c in range(2):
                    g = gp.tile([128, N], F32, tag="g")
                    nc.gpsimd.indirect_dma_start(
                        out=g[:], out_offset=None, in_=attn_d[:],
                        in_offset=IndirectOffsetOnAxis(ap=idxi[:, 2 * h + c:2 * h + c + 1], axis=0),
                    )
                    nc.sync.dma_start(out=oattn_d[:][h, c * 128:(c + 1) * 128, :], in_=g[:])
            gl = gp.tile([H, N], F32, tag="gl")
            nc.gpsimd.indirect_dma_start(
                out=gl[:], out_offset=None, in_=attn_d[:],
                in_offset=IndirectOffsetOnAxis(ap=lasti[:, 0:1], axis=0),
            )
            nc.sync.dma_start(out=oattn_d[:][:, K, :], in_=gl[:])

    nc.finalize()
    return nc


_NC = None


def _get_nc():
    global _NC
    if _NC is None:
        _NC = _build()
    return _NC


def _run(attn, value, mask, gumbel, trace=False):
    attn = np.ascontiguousarray(np.asarray(attn, dtype=np.float32))
    value = np.ascontiguousarray(np.asarray(value, dtype=np.float32))
    gumbel = np.ascontiguousarray(np.asarray(gumbel, dtype=np.float32))
    mask_u8 = np.ascontiguousarray(np.asarray(mask).astype(np.uint8))

    in_maps = [
        {
            "attn": attn[b].reshape(NH, N),
            "value": value[b].reshape(H, N * D),
            "maskp": mask_u8[b],
            "gumbel": gumbel[b],
        }
        for b in range(B)
    ]
    nc = _get_nc()
    res = run_bass_kernel_spmd(nc, in_maps, list(range(B)), trace=trace)

    new_attn = np.stack([np.asarray(res.results[b]["out_attn"]) for b in range(B)])
    unique_ids = np.stack([np.asarray(res.results[b]["out_ids"]) for b in range(B)])
    new_mask = np.stack([np.asarray(res.results[b]["out_mask"]) for b in range(B)]).astype(bool)
    return (new_attn, new_mask, unique_ids.astype(np.int32)), res


def kernel(attn, value, mask, gumbel):
    out, _ = _run(attn, value, mask, gumbel, trace=False)
    return out
